# revision 22
# baseline (speedup 1.0000x reference)
"""Lookahead depthwise convolution on 8 Trainium2 NeuronCores.

out[t, b, f] = sum_{c=0..K-1} x[t+c, b, f] * weight[f, c], zero-padded at the
right edge. x: (2048, 32, 1280) fp32, weight: (1280, 81) fp32.

Strategy: shard the (fully independent) feature dim across 8 cores, 160
features each. Per feature the time conv is a banded Toeplitz matmul over
128-wide time tiles:
  out[p, (blk, b)]  = sum_m  A_f[m, p]  * x[m,  (blk,   b)]   (intra-block)
                    + sum_m' B_f[m', p] * x[m', (blk+1, b)]   (lookahead tail)
with A_f[m, p] = w[f, m - p] (0 <= m-p <= 80) over all 128 rows and
B_f[m', p] = w[f, m' + 128 - p] (nonzero only for m' < 80).

Key layout choice: x is pre-arranged on the host as [t_in=128, (f, blk, b)]
fp16 so that ONE matmul per feature covers all 16 time blocks x 32 batch =
512 free columns (a full PSUM bank), with a second 480-column matmul adding
the lookahead tail from the next block. 2 weight loads + 2 matmuls per
feature (vs 32 tiny matmuls), LDWEIGHTS hidden behind streaming, fp16 I/O
halves HBM traffic. PSUM is evicted fp32->fp16 alternating Vector/Scalar.
fp16 x fp16 products are exact in the fp32 PSUM accumulator, so the error is
fp16 input/output rounding (~1e-3 rel).
"""

import numpy as np

import concourse.bass as bass
import concourse.bacc as bacc
import concourse.mybir as mybir
from concourse import tile
from concourse.bass_utils import run_bass_kernel_spmd

S, B, F, K = 2048, 32, 1280, 81
N_CORES = 8
FC = F // N_CORES          # features per core (160)
TB = S // 128              # time blocks (16)
FCH = 16                   # max features per chunk (tile slot size)
# Small chunks at the head (compute starts sooner) and tail (shorter drain).
CHUNKS = [8, 8] + [16] * 8 + [8, 8]
OUTQ = 8                   # features per output DMA segment
BROWS = 80                 # nonzero contraction rows of the B band

_compiled = None


def _build_program():
    nc = bacc.Bacc("TRN2", target_bir_lowering=False, debug=False)
    f32, f16 = mybir.dt.float32, mybir.dt.float16

    x_in = nc.declare_dram_parameter("x", [128, FC * TB * B], f16,
                                     isOutput=False)
    bA_in = nc.declare_dram_parameter("bandsA", [128, FC * 128], f16,
                                      isOutput=False)
    bB_in = nc.declare_dram_parameter("bandsB", [BROWS, FC * 128], f16,
                                      isOutput=False)
    out_ext = nc.declare_dram_parameter("out", [128, FC * TB * B], f16,
                                        isOutput=True)

    CW = TB * B            # columns per feature (512)

    with tile.TileContext(nc) as tc:
        with (
            tc.tile_pool(name="bandsA", bufs=5) as bApool,
            tc.tile_pool(name="bandsB", bufs=5) as bBpool,
            tc.tile_pool(name="xchunk", bufs=6) as xpool,
            tc.tile_pool(name="stage", bufs=3) as spool,
            tc.tile_pool(name="psum", bufs=8, space="PSUM") as ppool,
        ):
            # Chunk feature offsets.
            offs = [0]
            for fch in CHUNKS:
                offs.append(offs[-1] + fch)
            ncnk = len(CHUNKS)

            # Two HWDGE issue queues, loads balanced ~24/26 MB, and band
            # prefetches ride with the stream that needs them so an
            # output DMA waiting on evictions never blocks input prefetch.
            # Sync: x[c], bandsB[c].  Scalar: bandsA[c], out halves.
            # (SWDGE/gpsimd drains ~6x slower — measured 63 GB/s — and a
            # single HWDGE ring caps at ~300 GB/s, so both rings share.)
            # Bands are streamed through small pooled tiles instead of one
            # resident tile: same bytes, but the freed SBUF deepens x
            # prefetch to 6 chunks to ride out input-ring jitter.
            for c, fch in enumerate(CHUNKS):
                f0 = offs[c]
                bsl = slice(f0 * 128, (f0 + fch) * 128)
                xt = xpool.tile([128, fch * CW], f16, tag="xt",
                                padded_shape=[128, FCH * CW])
                csl = slice(f0 * CW, (f0 + fch) * CW)
                nc.sync.dma_start(out=xt[:], in_=x_in[:, csl])
                bA = bApool.tile([128, fch * 128], f16, tag="bA",
                                 padded_shape=[128, FCH * 128])
                bB = bBpool.tile([BROWS, fch * 128], f16, tag="bB",
                                 padded_shape=[BROWS, FCH * 128])
                nc.sync.dma_start(out=bB[:], in_=bB_in[:, bsl])
                nc.scalar.dma_start(out=bA[:], in_=bA_in[:, bsl])

                st = spool.tile([128, fch * CW], f16, tag="st",
                                padded_shape=[128, FCH * CW])
                xv = xt.rearrange("t (f n) -> t f n", n=CW)
                sv = st.rearrange("t (f n) -> t f n", n=CW)

                def send_out(lo, hi, seg):
                    # Tail chunks: nothing left on the Sync ring, so spread
                    # the final outs across both rings.
                    eng = (nc.sync if c >= ncnk - 2 and seg % 2 == 0
                           else nc.scalar)
                    eng.dma_start(
                        out=out_ext[:, (f0 + lo) * CW:(f0 + hi) * CW],
                        in_=st[:, lo * CW:hi * CW])

                seg = 0
                for j in range(fch):
                    ps = ppool.tile([128, CW], f32, tag="ps")
                    nc.tensor.matmul(
                        out=ps[:],
                        lhsT=bA[:, j * 128:(j + 1) * 128],
                        rhs=xv[:, j, :],
                        start=True, stop=False)
                    nc.tensor.matmul(
                        out=ps[:, 0:CW - B],
                        lhsT=bB[:, j * 128:(j + 1) * 128],
                        rhs=xv[0:BROWS, j, B:CW],
                        start=False, stop=True)
                    if j % 2 == 0:
                        nc.vector.tensor_copy(out=sv[:, j, :], in_=ps[:])
                    else:
                        nc.scalar.copy(out=sv[:, j, :], in_=ps[:])
                    if j == fch // 2 - 1:
                        send_out(0, fch // 2, 0)
                send_out(fch // 2, fch, 1)
    nc.finalize()
    return nc


def _build_bands(weight):
    """bandsA[m, f, p] = w[f, m - p]; bandsB[m', f, p] = w[f, m' + 128 - p]."""
    w16 = weight.astype(np.float16)
    m = np.arange(128)[:, None]
    p = np.arange(128)[None, :]
    dA = m - p
    mA = (dA >= 0) & (dA < K)
    A = np.where(mA[None], w16[:, np.clip(dA, 0, K - 1)], np.float16(0))
    mb_ = np.arange(BROWS)[:, None]
    dB = mb_ + 128 - p
    mB = (dB >= 0) & (dB < K)
    Bm = np.where(mB[None], w16[:, np.clip(dB, 0, K - 1)], np.float16(0))
    # [f, m, p] -> [m, f, p]
    return A.transpose(1, 0, 2), Bm.transpose(1, 0, 2)


def _prepare_in_maps(x, weight):
    x16 = x.astype(np.float16)                      # (S, B, F)
    A, Bm = _build_bands(weight)                    # [128, F, 128], [80, F, 128]
    in_maps = []
    for c in range(N_CORES):
        fl = slice(c * FC, (c + 1) * FC)
        # (S, B, FC) -> (blk, t_in, b, f) -> (t_in, f, blk, b)
        xc = x16[:, :, fl].reshape(TB, 128, B, FC).transpose(1, 3, 0, 2)
        in_maps.append({
            "x": np.ascontiguousarray(xc).reshape(128, FC * TB * B),
            "bandsA": np.ascontiguousarray(A[:, fl, :]).reshape(128, FC * 128),
            "bandsB": np.ascontiguousarray(Bm[:, fl, :]).reshape(BROWS, FC * 128),
        })
    return in_maps


def _assemble_output(results):
    outs = []
    for c in range(N_CORES):
        oc = np.asarray(results[c]["out"]).reshape(128, FC, TB, B)
        # (t_in, f, blk, b) -> (blk, t_in, b, f) -> (S, B, FC)
        outs.append(oc.transpose(2, 0, 3, 1).reshape(S, B, FC))
    return np.concatenate(outs, axis=2).astype(np.float32)


def kernel(x, weight):
    global _compiled
    x = np.asarray(x, dtype=np.float32)
    weight = np.asarray(weight, dtype=np.float32)
    if _compiled is None:
        _compiled = _build_program()
    in_maps = _prepare_in_maps(x, weight)
    res = run_bass_kernel_spmd(_compiled, in_maps, list(range(N_CORES)))
    return _assemble_output(res.results)


# revision 24
# speedup vs baseline: 1.1663x; 1.1663x over previous
"""Lookahead depthwise convolution on 8 Trainium2 NeuronCores.

out[t, b, f] = sum_{c=0..K-1} x[t+c, b, f] * weight[f, c], zero-padded at the
right edge. x: (2048, 32, 1280) fp32, weight: (1280, 81) fp32.

Strategy: shard the (fully independent) feature dim across 8 cores, 160
features each. Per feature the time conv is a banded Toeplitz matmul over
128-wide time tiles:
  out[p, (blk, b)]  = sum_m  A_f[m, p]  * x[m,  (blk,   b)]   (intra-block)
                    + sum_m' B_f[m', p] * x[m', (blk+1, b)]   (lookahead tail)
with A_f[m, p] = w[f, m - p] (0 <= m-p <= 80) over all 128 rows and
B_f[m', p] = w[f, m' + 128 - p] (nonzero only for m' < 80).

Key layout choice: x is pre-arranged on the host as [t_in=128, (f, blk, b)]
fp16 so that ONE matmul per feature covers all 16 time blocks x 32 batch =
512 free columns (a full PSUM bank), with a second 480-column matmul adding
the lookahead tail from the next block. 2 weight loads + 2 matmuls per
feature (vs 32 tiny matmuls), LDWEIGHTS hidden behind streaming, fp16 I/O
halves HBM traffic. PSUM is evicted fp32->fp16 alternating Vector/Scalar.
fp16 x fp16 products are exact in the fp32 PSUM accumulator, so the error is
fp16 input/output rounding (~1e-3 rel).
"""

import numpy as np

import concourse.bass as bass
import concourse.bacc as bacc
import concourse.mybir as mybir
from concourse import tile
from concourse.bass_utils import run_bass_kernel_spmd

S, B, F, K = 2048, 32, 1280, 81
N_CORES = 8
FC = F // N_CORES          # features per core (160)
TB = S // 128              # time blocks (16)
FCH = 16                   # max features per chunk (tile slot size)
# Small chunks at the head (compute starts sooner) and tail (shorter drain).
CHUNKS = [8, 8] + [16] * 8 + [8, 8]
OUTQ = 8                   # features per output DMA segment
BROWS = 80                 # nonzero contraction rows of the B band

_compiled = None


def _build_program():
    nc = bacc.Bacc("TRN2", target_bir_lowering=False, debug=False)
    f32, f16 = mybir.dt.float32, mybir.dt.float16

    x_in = nc.declare_dram_parameter("x", [128, FC * TB * B], f16,
                                     isOutput=False)
    bA_in = nc.declare_dram_parameter("bandsA", [128, FC * 128], f16,
                                      isOutput=False)
    bB_in = nc.declare_dram_parameter("bandsB", [BROWS, FC * 128], f16,
                                      isOutput=False)
    out_ext = nc.declare_dram_parameter("out", [128, FC * TB * B], f16,
                                        isOutput=True)

    CW = TB * B            # columns per feature (512)

    with tile.TileContext(nc) as tc:
        with (
            tc.tile_pool(name="bandsA", bufs=1) as bApool,
            tc.tile_pool(name="bandsB", bufs=1) as bBpool,
            tc.tile_pool(name="xchunk", bufs=4) as xpool,
            tc.tile_pool(name="stage", bufs=3) as spool,
            tc.tile_pool(name="psum", bufs=8, space="PSUM") as ppool,
        ):
            bA = bApool.tile([128, FC * 128], f16)
            bB = bBpool.tile([BROWS, FC * 128], f16)

            # Chunk feature offsets.
            offs = [0]
            for fch in CHUNKS:
                offs.append(offs[-1] + fch)
            ncnk = len(CHUNKS)

            def bsl(c):
                return slice(offs[c] * 128, offs[c + 1] * 128)

            # Two HWDGE issue queues, loads balanced ~24/26 MB, and band
            # prefetches ride ahead of the stream that needs them so an
            # output DMA waiting on evictions never blocks input prefetch.
            # Sync: bandsB[c+1], x[c] (in halves).  Scalar: bandsA[c+2],
            # out halves. (SWDGE/gpsimd drains ~6x slower — measured
            # 63 GB/s — and a single HWDGE ring caps at ~300 GB/s, so
            # both rings share.)
            nc.scalar.dma_start(out=bA[:, bsl(0)], in_=bA_in[:, bsl(0)])
            nc.scalar.dma_start(out=bA[:, bsl(1)], in_=bA_in[:, bsl(1)])
            nc.sync.dma_start(out=bB[:, bsl(0)], in_=bB_in[:, bsl(0)])

            for c, fch in enumerate(CHUNKS):
                f0 = offs[c]
                if c + 1 < ncnk:
                    nc.sync.dma_start(out=bB[:, bsl(c + 1)],
                                      in_=bB_in[:, bsl(c + 1)])
                xt = xpool.tile([128, fch * CW], f16, tag="xt",
                                padded_shape=[128, FCH * CW])
                # x in halves: the chunk's first matmuls start while the
                # second half is still streaming.
                fh = fch // 2
                nc.sync.dma_start(
                    out=xt[:, 0:fh * CW],
                    in_=x_in[:, f0 * CW:(f0 + fh) * CW])
                nc.sync.dma_start(
                    out=xt[:, fh * CW:fch * CW],
                    in_=x_in[:, (f0 + fh) * CW:(f0 + fch) * CW])
                if c + 2 < ncnk:
                    nc.scalar.dma_start(out=bA[:, bsl(c + 2)],
                                        in_=bA_in[:, bsl(c + 2)])

                st = spool.tile([128, fch * CW], f16, tag="st",
                                padded_shape=[128, FCH * CW])
                xv = xt.rearrange("t (f n) -> t f n", n=CW)
                sv = st.rearrange("t (f n) -> t f n", n=CW)

                def send_out(lo, hi, seg):
                    # Tail chunks: nothing left on the Sync ring, so spread
                    # the final outs across both rings.
                    eng = (nc.sync if c >= ncnk - 2 and seg % 2 == 0
                           else nc.scalar)
                    eng.dma_start(
                        out=out_ext[:, (f0 + lo) * CW:(f0 + hi) * CW],
                        in_=st[:, lo * CW:hi * CW])

                seg = 0
                for j in range(fch):
                    f = f0 + j
                    ps = ppool.tile([128, CW], f32, tag="ps")
                    nc.tensor.matmul(
                        out=ps[:],
                        lhsT=bA[:, f * 128:(f + 1) * 128],
                        rhs=xv[:, j, :],
                        start=True, stop=False)
                    nc.tensor.matmul(
                        out=ps[:, 0:CW - B],
                        lhsT=bB[:, f * 128:(f + 1) * 128],
                        rhs=xv[0:BROWS, j, B:CW],
                        start=False, stop=True)
                    if j % 2 == 0:
                        nc.vector.tensor_copy(out=sv[:, j, :], in_=ps[:])
                    else:
                        nc.scalar.copy(out=sv[:, j, :], in_=ps[:])
                    if j == fch // 2 - 1:
                        send_out(0, fch // 2, 0)
                send_out(fch // 2, fch, 1)
    nc.finalize()
    return nc


def _build_bands(weight):
    """bandsA[m, f, p] = w[f, m - p]; bandsB[m', f, p] = w[f, m' + 128 - p]."""
    w16 = weight.astype(np.float16)
    m = np.arange(128)[:, None]
    p = np.arange(128)[None, :]
    dA = m - p
    mA = (dA >= 0) & (dA < K)
    A = np.where(mA[None], w16[:, np.clip(dA, 0, K - 1)], np.float16(0))
    mb_ = np.arange(BROWS)[:, None]
    dB = mb_ + 128 - p
    mB = (dB >= 0) & (dB < K)
    Bm = np.where(mB[None], w16[:, np.clip(dB, 0, K - 1)], np.float16(0))
    # [f, m, p] -> [m, f, p]
    return A.transpose(1, 0, 2), Bm.transpose(1, 0, 2)


def _prepare_in_maps(x, weight):
    x16 = x.astype(np.float16)                      # (S, B, F)
    A, Bm = _build_bands(weight)                    # [128, F, 128], [80, F, 128]
    in_maps = []
    for c in range(N_CORES):
        fl = slice(c * FC, (c + 1) * FC)
        # (S, B, FC) -> (blk, t_in, b, f) -> (t_in, f, blk, b)
        xc = x16[:, :, fl].reshape(TB, 128, B, FC).transpose(1, 3, 0, 2)
        in_maps.append({
            "x": np.ascontiguousarray(xc).reshape(128, FC * TB * B),
            "bandsA": np.ascontiguousarray(A[:, fl, :]).reshape(128, FC * 128),
            "bandsB": np.ascontiguousarray(Bm[:, fl, :]).reshape(BROWS, FC * 128),
        })
    return in_maps


def _assemble_output(results):
    outs = []
    for c in range(N_CORES):
        oc = np.asarray(results[c]["out"]).reshape(128, FC, TB, B)
        # (t_in, f, blk, b) -> (blk, t_in, b, f) -> (S, B, FC)
        outs.append(oc.transpose(2, 0, 3, 1).reshape(S, B, FC))
    return np.concatenate(outs, axis=2).astype(np.float32)


def kernel(x, weight):
    global _compiled
    x = np.asarray(x, dtype=np.float32)
    weight = np.asarray(weight, dtype=np.float32)
    if _compiled is None:
        _compiled = _build_program()
    in_maps = _prepare_in_maps(x, weight)
    res = run_bass_kernel_spmd(_compiled, in_maps, list(range(N_CORES)))
    return _assemble_output(res.results)
